# revision 1
# baseline (speedup 1.0000x reference)
"""Batched zero-phase Butterworth lowpass (filtfilt) on Trainium2.

The per-row map x -> y is linear; the two-sided impulse response g decays
as r^|d| (r ~ 0.82), negligible beyond |d| = 64. So each row is computed as
a banded Toeplitz matmul: output tiles of 256 samples are produced by 3
accumulating PE matmuls against 128-sample transposed input windows, with
exact edge matrices for the first/last tile (built numerically on host from
b, a, zi). Rows are sharded 512 per NeuronCore across 8 cores.
"""

import sys

for _p in ("/opt/trn_rl_repo",):
    if _p not in sys.path:
        sys.path.insert(0, _p)

import numpy as np

import concourse.bass as bass
import concourse.tile as tile
from concourse import bacc
from concourse import mybir
from concourse.bass_utils import run_bass_kernel_spmd

N = 8192
ROWS = 4096
NCORES = 8
RPC = ROWS // NCORES          # 512 rows per core
GROUPS = RPC // 128           # 4 groups of 128 rows
F = 256                       # output tile width
NT = N // F                   # 32 output tiles per row
W = 384                       # edge window with exact operator columns
PADLEN = 18
USE_F32R = True

_mats_cache = {}
_nc_cache = {}


def _lfilter_batch(b, a, X, Zi):
    z = Zi.copy()
    Y = np.empty_like(X)
    b1, bm, bl = b[0], b[1:-1], b[-1]
    am, al = a[1:-1], a[-1]
    for t in range(X.shape[1]):
        xt = X[:, t]
        y = b1 * xt + z[:, 0]
        Y[:, t] = y
        z[:, :-1] = z[:, 1:] + np.outer(xt, bm) - np.outer(y, am)
        z[:, -1] = bl * xt - al * y
    return Y


def _filtfilt_batch(b, a, zi, X):
    left = 2 * X[:, :1] - X[:, PADLEN:0:-1]
    right = 2 * X[:, -1:] - X[:, -2:-(PADLEN + 2):-1]
    ext = np.concatenate([left, X, right], axis=1)
    y = _lfilter_batch(b, a, ext, np.outer(ext[:, 0], zi))
    y = _lfilter_batch(b, a, y[:, ::-1], np.outer(y[:, -1], zi))[:, ::-1]
    return y[:, PADLEN:-PADLEN]


def _build_mats(b, a, zi):
    """9 rhs matrices [128, 256] f32: first-tile specials, interior band,
    last-tile specials."""
    key = (b.tobytes(), a.tobytes(), zi.tobytes())
    if key in _mats_cache:
        return _mats_cache[key]
    b64, a64, zi64 = (np.asarray(v, np.float64) for v in (b, a, zi))

    j0 = N // 2
    basis = np.zeros((2 * W + 1, N))
    for i in range(W):
        basis[i, i] = 1.0
        basis[W + i, N - W + i] = 1.0
    basis[2 * W, j0] = 1.0
    cols = _filtfilt_batch(b64, a64, zi64, basis)
    g = cols[2 * W]                # M[:, j0]; band value g[d] = g[j0 + d]
    Mleft = cols[:W].T             # [N, W]  M[t, j], j < W
    Mright = cols[W:2 * W].T       # [N, W]  M[t, N - W + j]

    def gband(d):
        dd = np.clip(j0 + d, 0, N - 1)
        out = g[dd]
        out[np.abs(d) > 150] = 0.0
        return out

    c = np.arange(128)[:, None]
    f = np.arange(F)[None, :]
    G = [gband(128 * w - 64 + c - f) for w in range(3)]

    def special(tile_idx):
        R = []
        t1 = F * tile_idx
        tt = t1 + np.arange(F)[None, :]
        for w in range(3):
            jj = np.broadcast_to(t1 - 64 + 128 * w + c, (128, F))
            valid = (jj >= 0) & (jj < N)
            jcl = np.clip(jj, 0, N - 1)
            use_left = jcl < W
            use_right = jcl >= N - W
            band = gband(jj - tt)
            lw = Mleft[np.broadcast_to(tt, jj.shape), np.where(use_left, jcl, 0)]
            rw = Mright[np.broadcast_to(tt, jj.shape),
                        np.where(use_right, jcl - (N - W), 0)]
            Rw = np.where(use_left, lw, np.where(use_right, rw, band))
            Rw[~valid] = 0.0
            R.append(Rw)
        return R

    wts = np.stack(special(0) + G + special(NT - 1)).astype(np.float32)
    wts = np.ascontiguousarray(wts)
    _mats_cache[key] = wts
    return wts


def _build_nc():
    if "nc" in _nc_cache:
        return _nc_cache["nc"]
    f32 = mybir.dt.float32
    f32r = mybir.dt.float32r
    mmdt = f32r if USE_F32R else f32
    nc = bacc.Bacc()
    x_in = nc.declare_dram_parameter("x", [RPC, N], f32, isOutput=False)
    wts_in = nc.declare_dram_parameter("wts", [9, 128, F], mmdt, isOutput=False)
    idn_in = nc.declare_dram_parameter("idn", [128, 128], f32, isOutput=False)
    y_out = nc.declare_dram_parameter("y", [RPC, N], f32, isOutput=True)

    with tile.TileContext(nc) as tc:
        with (
            tc.tile_pool(name="const", bufs=1) as constp,
            tc.tile_pool(name="xp", bufs=2) as xp,
            tc.tile_pool(name="xtp", bufs=8) as xtp,
            tc.tile_pool(name="outp", bufs=2) as outp,
            tc.tile_pool(name="pst", bufs=4, space="PSUM") as pst,
            tc.tile_pool(name="psc", bufs=4, space="PSUM") as psc,
        ):
            ident = constp.tile([128, 128], f32, tag="ident")
            nc.sync.dma_start(ident[:, :], idn_in[:, :])
            wt_all = constp.tile([128, 9 * F], mmdt, tag="wt_all")
            nc.sync.dma_start(
                wt_all[:, :].rearrange("p (i f) -> p i f", i=9),
                wts_in.ap().rearrange("i p f -> p i f"),
            )
            wtiles = [wt_all[:, i * F:(i + 1) * F] for i in range(9)]

            for gidx in range(GROUPS):
                xpad = xp.tile([128, 64 + N + 64], f32, tag="xpad")
                nc.gpsimd.memset(xpad[:, 0:64], 0.0)
                nc.gpsimd.memset(xpad[:, 64 + N:], 0.0)
                nc.sync.dma_start(
                    xpad[:, 64:64 + N],
                    x_in[gidx * 128:(gidx + 1) * 128, :],
                )
                outbuf = outp.tile([128, N], f32, tag="outbuf")
                xt_tiles = {}
                for k in range(NT):
                    for j in (2 * k, 2 * k + 1, 2 * k + 2):
                        if j not in xt_tiles:
                            pt = pst.tile([128, 128], f32, tag="pt")
                            nc.tensor.transpose(
                                pt[:, :], xpad[:, 128 * j:128 * (j + 1)],
                                ident[:, :],
                            )
                            xt = xtp.tile([128, 128], mmdt, tag="xt")
                            nc.vector.tensor_copy(xt[:, :], pt[:, :])
                            xt_tiles[j] = xt
                    pc = psc.tile([128, F], f32, tag="pc")
                    base = 0 if k == 0 else (6 if k == NT - 1 else 3)
                    for w in range(3):
                        nc.tensor.matmul(
                            pc[:, :], xt_tiles[2 * k + w][:, :],
                            wtiles[base + w],
                            start=(w == 0), stop=(w == 2),
                        )
                    # split PSUM->SBUF drain between DVE and ACT to balance load
                    if k % 2 == 0:
                        nc.vector.tensor_copy(
                            outbuf[:, F * k:F * (k + 1)], pc[:, :]
                        )
                    else:
                        nc.scalar.copy(outbuf[:, F * k:F * (k + 1)], pc[:, :])
                # store on the ACT HWDGE ring so it overlaps the SP-ring loads
                nc.scalar.dma_start(
                    y_out[gidx * 128:(gidx + 1) * 128, :], outbuf[:, :]
                )
    nc.compile()
    _nc_cache["nc"] = nc
    return nc


def _run(inputs, trace=False, trace_kwargs=None):
    x = np.ascontiguousarray(np.asarray(inputs["x"], np.float32))
    b = np.asarray(inputs["b"], np.float32)
    a = np.asarray(inputs["a"], np.float32)
    zi = np.asarray(inputs["zi"], np.float32)
    wts = _build_mats(b, a, zi)
    idn = np.eye(128, dtype=np.float32)
    nc = _build_nc()
    in_maps = [
        {"x": x[i * RPC:(i + 1) * RPC], "wts": wts, "idn": idn}
        for i in range(NCORES)
    ]
    res = run_bass_kernel_spmd(
        nc, in_maps, list(range(NCORES)), trace=trace,
        **(trace_kwargs or {}),
    )
    y = np.concatenate([res.results[i]["y"] for i in range(NCORES)], axis=0)
    return y, res


def kernel(**inputs) -> np.ndarray:
    y, _ = _run(inputs, trace=False)
    return y

